# revision 1
# baseline (speedup 1.0000x reference)
"""Trainium2 Bass kernel for nn_CrossAttention_51539607552970.

Sharding: 8 cores = 2 (batch) x 4 (GQA kv-head groups). Each core computes
4 query heads + its single kv head for one batch element, producing a
partial output (its head-group's contribution through wo); the host sums
the 4 partials per batch element (tensor-parallel unshard).

On-device layout is feature-major: the host passes x/c transposed
([hid, tokens]) so every matmul contracts the partition dimension
natively.  Scores are computed transposed ([keys, q]) so the softmax
denominator is a PE ones-matmul that accumulates into the same PSUM bank
as the P@V accumulator (columns 256:512 vs 0:256).  All matmuls run in
float32r (full-rate fp32 mode, ~1e-4 precision).
"""

import sys

sys.path.insert(0, "/opt/trn_rl_repo")

import numpy as np

import concourse.bass as bass
import concourse.mybir as mybir
import concourse.tile as tile
from concourse import bacc
from concourse.bass_utils import run_bass_kernel_spmd
from concourse.masks import make_identity

F32 = mybir.dt.float32
F32R = mybir.dt.float32r
BF16 = mybir.dt.bfloat16
AF = mybir.ActivationFunctionType
OP = mybir.AluOpType

# Problem constants (hardcoded per contract).
B, S, L = 2, 2048, 2048
H, KVH, D = 16, 4, 128
HID = H * D
EPS = 1e-6
SCALE = 1.0 / np.sqrt(D)

NH = 4           # query heads per core
P = 128          # partitions
HC = HID // P    # 16 hid chunks
KC = L // P      # 16 key chunks
PB = 512         # projection block width (tokens)
AB = 512           # attention block width (queries)
NPB = S // PB    # 4
NAB = S // AB    # 4

_compiled = None


def _build():
    nc = bacc.Bacc("TRN2", num_devices=8)

    xT = nc.dram_tensor("xT", [HID, S], F32R, kind="ExternalInput")
    cT = nc.dram_tensor("cT", [HID, L], F32R, kind="ExternalInput")
    wq = nc.dram_tensor("wq", [HID, NH * D], F32R, kind="ExternalInput")
    wk = nc.dram_tensor("wk", [HID, D], F32R, kind="ExternalInput")
    wv = nc.dram_tensor("wv", [HID, D], F32R, kind="ExternalInput")
    wo = nc.dram_tensor("wo", [NH * D, HID], F32R, kind="ExternalInput")
    nqw = nc.dram_tensor("nqw", [P, 1], F32, kind="ExternalInput")
    nkw = nc.dram_tensor("nkw", [P, 1], F32, kind="ExternalInput")
    out = nc.dram_tensor("out", [S, HID], F32, kind="ExternalOutput")

    with nc.allow_low_precision(reason="f32r matmul input rounding"), \
         tile.TileContext(nc) as tc:
        with tc.tile_pool(name="consts", bufs=1) as consts, \
             tc.tile_pool(name="weights", bufs=1) as weights, \
             tc.tile_pool(name="stream", bufs=12) as stream, \
             tc.tile_pool(name="kv", bufs=1) as kvpool, \
             tc.tile_pool(name="xqt", bufs=1) as xqtpool, \
             tc.tile_pool(name="small", bufs=2) as small, \
             tc.tile_pool(name="esbp", bufs=4) as esbp, \
             tc.tile_pool(name="outp", bufs=3) as outp, \
             tc.tile_pool(name="psum", bufs=1, space="PSUM") as psum:

            # ---- constants ----
            ones_f = consts.tile([P, P], F32)
            nc.vector.memset(ones_f[:], 1.0)
            ones = consts.tile([P, P], F32R)
            nc.scalar.copy(ones[:], ones_f[:])
            ones_b = consts.tile([P, P], BF16)
            nc.scalar.copy(ones_b[:], ones_f[:])
            ident = consts.tile([P, P], F32)
            make_identity(nc, ident)
            nqw_sb = consts.tile([P, 1], F32)
            nc.sync.dma_start(nqw_sb[:], nqw[:])
            nkw_sb = consts.tile([P, 1], F32)
            nc.sync.dma_start(nkw_sb[:], nkw[:])
            eps_sb = consts.tile([P, 1], F32)
            nc.vector.memset(eps_sb[:], EPS)

            # ---- resident weights ----
            wq_sb = weights.tile([P, HC * NH * D], F32R)   # 16 chunks x 512
            for hc in range(HC):
                nc.sync.dma_start(wq_sb[:, hc * 512:(hc + 1) * 512],
                                  wq[hc * P:(hc + 1) * P, :])
            wk_sb = weights.tile([P, HC * D], F32R)
            wv_sb = weights.tile([P, HC * D], F32R)
            for hc in range(HC):
                nc.sync.dma_start(wk_sb[:, hc * D:(hc + 1) * D],
                                  wk[hc * P:(hc + 1) * P, :])
                nc.sync.dma_start(wv_sb[:, hc * D:(hc + 1) * D],
                                  wv[hc * P:(hc + 1) * P, :])
            wo_sb = weights.tile([P, NH * HID], F32R)      # 4 head-chunks x 2048
            for h in range(NH):
                nc.sync.dma_start(wo_sb[:, h * HID:(h + 1) * HID],
                                  wo[h * P:(h + 1) * P, :])

            # ---- persistent activations ----
            kT_sb = kvpool.tile([P, L], BF16)              # [D, keys]
            v_sb = kvpool.tile([P, KC * D], BF16)          # kt-th block = [keys(kt), D]
            xqT_list = [xqtpool.tile([P, S], BF16, name=f"xqT{h}") for h in range(NH)]

            # =========== Phase B: K/V projections (stream cT) ===========
            for kcol in range(4):  # 512-wide key column blocks
                ct_tiles = []
                for hc in range(HC):
                    t = stream.tile([P, 512], F32R, name="ct", tag="stream")
                    nc.sync.dma_start(
                        t[:], cT[hc * P:(hc + 1) * P, kcol * 512:(kcol + 1) * 512])
                    ct_tiles.append(t)

                # kT/vT blocks: [D, 512 keys]; interleave per-chunk so each
                # streamed cT tile is consumed immediately by both matmuls.
                kps = psum.tile([P, 512], F32, name="kps", tag="combo", bufs=4)
                vps = psum.tile([P, 512], F32, name="vps", tag="combo", bufs=4)
                for hc in range(HC):
                    nc.tensor.matmul(kps[:], wk_sb[:, hc * D:(hc + 1) * D],
                                     ct_tiles[hc][:],
                                     start=(hc == 0), stop=(hc == HC - 1))
                    nc.tensor.matmul(vps[:], wv_sb[:, hc * D:(hc + 1) * D],
                                     ct_tiles[hc][:],
                                     start=(hc == 0), stop=(hc == HC - 1))
                vT_sb = small.tile([P, 512], F32, name="vT", tag="vT")
                nc.vector.tensor_copy(vT_sb[:], vps[:])
                # k rmsnorm over D (partition dim): sumsq via ones matmul
                ksq = small.tile([P, 512], F32R, name="ksq", tag="sq")
                nc.scalar.square(ksq[:], kps[:])
                ksum = psum.tile([P, 512], F32, name="ksum", tag="work", bufs=2)
                nc.tensor.matmul(ksum[:], ones[:], ksq[:], start=True, stop=True)
                krs = small.tile([P, 512], F32, name="krs", tag="rs")
                nc.scalar.activation(krs[:], ksum[:], AF.Sqrt,
                                     bias=eps_sb[:], scale=1.0 / D)
                krr = small.tile([P, 512], F32, name="krr", tag="rr")
                nc.vector.reciprocal_approx_fast(out=krr[:], in_=krs[:])
                # kT = (kps * nkw) * rsqrt  (fused)
                nc.vector.scalar_tensor_tensor(
                    out=kT_sb[:, kcol * 512:(kcol + 1) * 512], in0=kps[:],
                    scalar=nkw_sb[:], in1=krr[:], op0=OP.mult, op1=OP.mult)
                # transpose 128x128 blocks -> v_sb [keys, D]
                for j in range(4):
                    kt = kcol * 4 + j
                    tp = psum.tile([P, P], F32, name="tp", tag="work", bufs=2)
                    nc.tensor.transpose(tp[:], vT_sb[:, j * P:(j + 1) * P], ident[:])
                    nc.vector.tensor_copy(v_sb[:, kt * D:(kt + 1) * D], tp[:])

            # =========== Phase A: Q projection (stream xT) ===========
            for pb in range(NPB):
                xt_tiles = []
                for hc in range(HC):
                    t = stream.tile([P, PB], F32R, name="xt", tag="stream")
                    nc.sync.dma_start(
                        t[:], xT[hc * P:(hc + 1) * P, pb * PB:(pb + 1) * PB])
                    xt_tiles.append(t)
                qpss = [psum.tile([P, PB], F32, name=f"qps{h}",
                                  tag="combo", bufs=4) for h in range(NH)]
                for hc in range(HC):
                    for h in range(NH):
                        nc.tensor.matmul(
                            qpss[h][:],
                            wq_sb[:, hc * 512 + h * D: hc * 512 + (h + 1) * D],
                            xt_tiles[hc][:],
                            start=(hc == 0), stop=(hc == HC - 1))
                for h in range(NH):
                    qps = qpss[h]
                    qsq = small.tile([P, PB], F32R, name="qsq", tag="sq")
                    nc.scalar.square(qsq[:], qps[:])
                    qsum = psum.tile([P, PB], F32, name="qsum", tag="work", bufs=2)
                    nc.tensor.matmul(qsum[:], ones[:], qsq[:], start=True, stop=True)
                    qrs = small.tile([P, PB], F32, name="qrs", tag="rs")
                    nc.scalar.activation(qrs[:], qsum[:], AF.Sqrt,
                                         bias=eps_sb[:], scale=1.0 / D)
                    qrr = small.tile([P, PB], F32, name="qrr", tag="rr")
                    nc.vector.reciprocal_approx_fast(out=qrr[:], in_=qrs[:])
                    nc.vector.scalar_tensor_tensor(
                        out=xqT_list[h][:, pb * PB:(pb + 1) * PB], in0=qps[:],
                        scalar=nqw_sb[:], in1=qrr[:], op0=OP.mult, op1=OP.mult)

            # =========== Phase C: attention + wo ===========
            for ab in range(NAB):
                q0 = ab * AB
                attn_sbs = []
                for hg in range(2):          # head groups of 2 (PSUM budget)
                    hs = [2 * hg, 2 * hg + 1]
                    attps = {h: psum.tile([P, AB], F32, name=f"attps{h}",
                                          tag="combo", bufs=4) for h in hs}
                    sumps = {h: psum.tile([P, AB], F32, name=f"sumps{h}",
                                          tag="work", bufs=2) for h in hs}
                    for kt in range(KC):
                        sts = {}
                        for h in hs:
                            st = psum.tile([P, AB], F32, name="st", tag="st",
                                           bufs=2)
                            nc.tensor.matmul(st[:],
                                             kT_sb[:, kt * P:(kt + 1) * P],
                                             xqT_list[h][:, q0:q0 + AB],
                                             start=True, stop=True)
                            sts[h] = st
                        es = {}
                        for h in hs:
                            e = esbp.tile([P, AB], BF16, name="e", tag="e")
                            nc.scalar.activation(e[:], sts[h][:], AF.Exp)
                            es[h] = e
                        for h in hs:
                            nc.tensor.matmul(sumps[h][:], ones_b[:], es[h][:],
                                             start=(kt == 0),
                                             stop=(kt == KC - 1))
                        for h in hs:
                            nc.tensor.matmul(attps[h][:],
                                             v_sb[:, kt * D:(kt + 1) * D],
                                             es[h][:],
                                             start=(kt == 0),
                                             stop=(kt == KC - 1))
                    for h in hs:
                        rr = small.tile([P, AB], F32, name="arr", tag="arr")
                        nc.vector.reciprocal_approx_fast(out=rr[:], in_=sumps[h][:])
                        attn = small.tile([P, AB], F32R, name="attn",
                                          tag=f"attn{h}", bufs=2)
                        nc.vector.tensor_tensor(
                            out=attn[:], in0=attps[h][:], in1=rr[:],
                            op=OP.mult)
                        attn_sbs.append((h, attn))
                attn_map = dict(attn_sbs)
                # wo: out[q, :] += attn_h^T @ wo_h for 128-row q-subtiles
                for qs in range(AB // P):  # 4
                    wops = [psum.tile([P, 512], F32, name=f"wop{ht}",
                                      tag="combo", bufs=4) for ht in range(4)]
                    for h in range(NH):
                        for ht in range(4):
                            nc.tensor.matmul(
                                wops[ht][:],
                                attn_map[h][:, qs * P:(qs + 1) * P],
                                wo_sb[:, h * HID + ht * 512: h * HID + (ht + 1) * 512],
                                start=(h == 0), stop=(h == NH - 1))
                    for ht in range(4):
                        ot = outp.tile([P, 512], F32, name="ot", tag="ot")
                        nc.vector.tensor_copy(ot[:], wops[ht][:])
                        nc.sync.dma_start(
                            out[q0 + qs * P: q0 + (qs + 1) * P,
                                ht * 512:(ht + 1) * 512], ot[:])

    nc.compile()
    return nc


def _get_compiled():
    global _compiled
    if _compiled is None:
        _compiled = _build()
    return _compiled


def _shard_inputs(x, c, wq, wkv, wo, norm_q_w, norm_k_w):
    x = np.asarray(x, np.float32)
    c = np.asarray(c, np.float32)
    wq = np.asarray(wq, np.float32)
    wkv = np.asarray(wkv, np.float32)
    wo = np.asarray(wo, np.float32)
    nqw = (np.asarray(norm_q_w, np.float32) * np.float32(SCALE)).reshape(P, 1)
    nkw = np.asarray(norm_k_w, np.float32).reshape(P, 1).copy()

    xTs = [np.ascontiguousarray(x[b].T) for b in range(B)]
    cTs = [np.ascontiguousarray(c[b].T) for b in range(B)]
    in_maps = []
    for core in range(8):
        b, g = core // 4, core % 4
        blk = wkv[:, g * 256:(g + 1) * 256]
        in_maps.append({
            "xT": xTs[b],
            "cT": cTs[b],
            "wq": np.ascontiguousarray(wq[:, g * 512:(g + 1) * 512]),
            "wk": np.ascontiguousarray(blk[:, 0::2]),
            "wv": np.ascontiguousarray(blk[:, 1::2]),
            "wo": np.ascontiguousarray(wo[g * 512:(g + 1) * 512, :]),
            "nqw": nqw,
            "nkw": nkw,
        })
    return in_maps


def run_sharded(inputs, trace=False, trace_cores=None):
    """Run the SPMD kernel; returns (full_output, BassKernelResults)."""
    nc = _get_compiled()
    in_maps = _shard_inputs(**inputs)
    res = run_bass_kernel_spmd(nc, in_maps, core_ids=list(range(8)),
                               trace=trace, trace_cores=trace_cores)
    parts = [r["out"] for r in res.results]
    full = np.empty((B, S, HID), np.float32)
    for b in range(B):
        full[b] = np.sum(np.stack([parts[4 * b + g] for g in range(4)], 0),
                         axis=0, dtype=np.float64).astype(np.float32)
    return full, res


def kernel(**inputs) -> np.ndarray:
    out, _ = run_sharded(inputs, trace=False)
    return out



# revision 5
# speedup vs baseline: 1.2447x; 1.2447x over previous
"""Trainium2 Bass kernel for nn_CrossAttention_51539607552970.

Sharding: 8 cores = 2 (batch) x 4 (GQA kv-head groups). Each core computes
4 query heads + its single kv head for one batch element, producing a
partial output (its head-group's contribution through wo); the host sums
the 4 partials per batch element (tensor-parallel unshard).

Host passes x/c transposed, pre-tiled ([blk][hc][128, 512] contiguous)
in bf16, and weights as SBUF images, so every DMA is contiguous.  All
matmuls run in bf16 (full PE rate, f32 PSUM accumulate).

Schedule: Q-projection (A) and KV-projection (B) phases interleave at
block level (A0 B0 A1 B1 ...) so compute starts ~1us in and DMA (the
pre-attention bottleneck) streams in need-time order.  Attention (C)
runs single-head pipelines; the output projection (wo) of the previous
query block is interleaved one matmul per kt step as PE filler so the
exp (scalar) latency never stalls the PE.  PSUM: attps 2 + sumps 1 +
st 3 + wo 1 + tp 1 = 8 banks.
"""

import sys

sys.path.insert(0, "/opt/trn_rl_repo")

import ml_dtypes
import numpy as np

import concourse.bass as bass
import concourse.mybir as mybir
import concourse.tile as tile
from concourse import bacc
from concourse.bass_utils import run_bass_kernel_spmd
from concourse.masks import make_identity

F32 = mybir.dt.float32
BF16 = mybir.dt.bfloat16
AF = mybir.ActivationFunctionType
OP = mybir.AluOpType

# Problem constants (hardcoded per contract).
B, S, L = 2, 2048, 2048
H, KVH, D = 16, 4, 128
HID = H * D
EPS = 1e-6
SCALE = 1.0 / np.sqrt(D)

NH = 4           # query heads per core
P = 128          # partitions
HC = HID // P    # 16 hid chunks
KC = L // P      # 16 key chunks
PB = 512         # projection block width (tokens)
AB = 512         # attention block width (queries)
NPB = S // PB    # 4
NAB = S // AB    # 4

_compiled = None


def _build():
    nc = bacc.Bacc("TRN2", num_devices=8)

    xT = nc.dram_tensor("xT", [NPB, HC, P, PB], BF16, kind="ExternalInput")
    cT = nc.dram_tensor("cT", [4, HC, P, 512], BF16, kind="ExternalInput")
    wq = nc.dram_tensor("wq", [HC, P, NH * D], BF16, kind="ExternalInput")
    wk = nc.dram_tensor("wk", [P, HC * D], BF16, kind="ExternalInput")
    wv = nc.dram_tensor("wv", [P, HC * D], BF16, kind="ExternalInput")
    wo = nc.dram_tensor("wo", [P, NH * HID], BF16, kind="ExternalInput")
    nqw = nc.dram_tensor("nqw", [P, 1], F32, kind="ExternalInput")
    nkw = nc.dram_tensor("nkw", [P, 1], F32, kind="ExternalInput")
    out = nc.dram_tensor("out", [S, HID], F32, kind="ExternalOutput")

    with nc.allow_low_precision(reason="bf16 matmul inputs"), \
         tile.TileContext(nc) as tc:
        with tc.tile_pool(name="consts", bufs=1) as consts, \
             tc.tile_pool(name="weights", bufs=1) as weights, \
             tc.tile_pool(name="stream", bufs=20) as stream, \
             tc.tile_pool(name="kv", bufs=1) as kvpool, \
             tc.tile_pool(name="xqt", bufs=1) as xqtpool, \
             tc.tile_pool(name="small", bufs=2) as small, \
             tc.tile_pool(name="esbp", bufs=4) as esbp, \
             tc.tile_pool(name="outp", bufs=3) as outp, \
             tc.tile_pool(name="psum", bufs=1, space="PSUM") as psum:

            # ---- constants (no DMA deps) ----
            ones_f = consts.tile([P, P], F32)
            nc.vector.memset(ones_f[:], 1.0)
            ones_b = consts.tile([P, P], BF16)
            nc.scalar.copy(ones_b[:], ones_f[:])
            ident_f = consts.tile([P, P], F32)
            make_identity(nc, ident_f)
            ident = consts.tile([P, P], BF16)
            nc.scalar.copy(ident[:], ident_f[:])
            eps_sb = consts.tile([P, 1], F32)
            nc.vector.memset(eps_sb[:], EPS)

            nqw_sb = consts.tile([P, 1], F32)
            nkw_sb = consts.tile([P, 1], F32)
            nc.sync.dma_start(nqw_sb[:], nqw[:])
            nc.sync.dma_start(nkw_sb[:], nkw[:])

            # ---- weights ----
            wq_cs = [weights.tile([P, NH * D], BF16, name=f"wqc{hc}")
                     for hc in range(HC)]
            wk_sb = weights.tile([P, HC * D], BF16)
            wv_sb = weights.tile([P, HC * D], BF16)
            wo_sb = weights.tile([P, NH * HID], BF16)

            # ---- persistent activations ----
            kT_sb = kvpool.tile([P, L], BF16)              # [D, keys]
            v_sb = kvpool.tile([P, KC * D], BF16)          # kt-th blk [keys, D]
            xqT_list = [xqtpool.tile([P, S], BF16, name=f"xqT{h}")
                        for h in range(NH)]

            # PSUM tags (static banks): catt 2 + csum 1 + st 3 + wop 1 + tp 1
            def t_catt(nm):
                return psum.tile([P, 512], F32, name=nm, tag="catt", bufs=2)

            def t_csum(nm):
                return psum.tile([P, 512], F32, name=nm, tag="csum", bufs=1)

            def t_st(nm):
                return psum.tile([P, 512], F32, name=nm, tag="st", bufs=3)

            def t_wop(nm):
                return psum.tile([P, 512], F32, name=nm, tag="wop", bufs=1)

            # ---------- phase-A block: Q projection for one pb ----------
            def emit_A(pb, xt_tiles, extra_pe=None):
                # two 2-head passes so the norm chain of pass1 overlaps
                # pass2's matmuls (and pass2's norm overlaps the next block)
                for hp in range(2):
                    hs = [2 * hp, 2 * hp + 1]
                    qpss = {h: (t_catt(f"qps{h}") if hp == 0
                                else t_csum(f"qps{h}") if h == 2
                                else t_st(f"qps{h}")) for h in hs}
                    for hc in range(HC):
                        for h in hs:
                            nc.tensor.matmul(
                                qpss[h][:],
                                wq_cs[hc][:, h * D:(h + 1) * D],
                                xt_tiles[hc][:],
                                start=(hc == 0), stop=(hc == HC - 1))
                        if extra_pe and hp == 0 and hc in (3, 7, 11, 15):
                            extra_pe(hc // 4)
                    for h in hs:
                        qps = qpss[h]
                        qsq = small.tile([P, PB], BF16, name="qsq", tag="sq")
                        nc.scalar.square(qsq[:], qps[:])
                        qsum = t_wop("qsum")
                        nc.tensor.matmul(qsum[:], ones_b[:], qsq[:],
                                         start=True, stop=True)
                        qrs = small.tile([P, PB], F32, name="qrs", tag="rs")
                        nc.scalar.activation(qrs[:], qsum[:], AF.Sqrt,
                                             bias=eps_sb[:], scale=1.0 / D)
                        qrr = small.tile([P, PB], F32, name="qrr", tag="rr")
                        nc.vector.reciprocal_approx_fast(out=qrr[:], in_=qrs[:])
                        nc.vector.scalar_tensor_tensor(
                            out=xqT_list[h][:, pb * PB:(pb + 1) * PB],
                            in0=qps[:], scalar=nqw_sb[:], in1=qrr[:],
                            op0=OP.mult, op1=OP.mult)

            # ---------- phase-B block: K/V projection for one kcol ----------
            # returns a closure emitting the 4 deferred V-transposes
            def emit_B(kcol):
                ct_tiles = []
                for hc in range(HC):
                    t = stream.tile([P, 512], BF16, name="ct", tag="stream")
                    nc.sync.dma_start(t[:], cT[kcol, hc])
                    ct_tiles.append(t)
                kps = t_st("kps")
                vps = t_st("vps")
                for hc in range(HC):
                    nc.tensor.matmul(kps[:], wk_sb[:, hc * D:(hc + 1) * D],
                                     ct_tiles[hc][:],
                                     start=(hc == 0), stop=(hc == HC - 1))
                    nc.tensor.matmul(vps[:], wv_sb[:, hc * D:(hc + 1) * D],
                                     ct_tiles[hc][:],
                                     start=(hc == 0), stop=(hc == HC - 1))
                vT_sb = small.tile([P, 512], BF16, name="vT", tag="vT")
                nc.vector.tensor_copy(vT_sb[:], vps[:])
                ksq = small.tile([P, 512], BF16, name="ksq", tag="sq")
                nc.scalar.square(ksq[:], kps[:])
                ksum = t_wop("ksum")
                nc.tensor.matmul(ksum[:], ones_b[:], ksq[:],
                                 start=True, stop=True)
                krs = small.tile([P, 512], F32, name="krs", tag="rs")
                nc.scalar.activation(krs[:], ksum[:], AF.Sqrt,
                                     bias=eps_sb[:], scale=1.0 / D)
                krr = small.tile([P, 512], F32, name="krr", tag="rr")
                nc.vector.reciprocal_approx_fast(out=krr[:], in_=krs[:])
                nc.vector.scalar_tensor_tensor(
                    out=kT_sb[:, kcol * 512:(kcol + 1) * 512], in0=kps[:],
                    scalar=nkw_sb[:], in1=krr[:], op0=OP.mult, op1=OP.mult)

                def transpose_one(j):
                    kt = kcol * 4 + j
                    tp = psum.tile([P, P], BF16, name="tp", tag="tp", bufs=1)
                    nc.tensor.transpose(tp[:], vT_sb[:, j * P:(j + 1) * P],
                                        ident[:])
                    nc.vector.tensor_copy(v_sb[:, kt * D:(kt + 1) * D], tp[:])
                return transpose_one

            # =========== interleaved A/B with need-ordered DMA ===========
            # A0 prerequisites stream first (wq chunk + xt0 chunk pairs)
            xt0_tiles = []
            for hc in range(HC):
                nc.sync.dma_start(wq_cs[hc][:], wq[hc])
                t = stream.tile([P, PB], BF16, name="xt", tag="xstream")
                nc.sync.dma_start(t[:], xT[0, hc])
                xt0_tiles.append(t)
            nc.sync.dma_start(wk_sb[:], wk[:])
            nc.sync.dma_start(wv_sb[:], wv[:])

            emit_A(0, xt0_tiles)
            tr0 = emit_B(0)
            xt1 = []
            for hc in range(HC):
                t = stream.tile([P, PB], BF16, name="xt", tag="xstream")
                nc.sync.dma_start(t[:], xT[1, hc])
                xt1.append(t)
            emit_A(1, xt1, extra_pe=tr0)
            tr1 = emit_B(1)
            xt2 = []
            for hc in range(HC):
                t = stream.tile([P, PB], BF16, name="xt", tag="xstream")
                nc.sync.dma_start(t[:], xT[2, hc])
                xt2.append(t)
            emit_A(2, xt2, extra_pe=tr1)
            tr2 = emit_B(2)
            xt3 = []
            for hc in range(HC):
                t = stream.tile([P, PB], BF16, name="xt", tag="xstream")
                nc.sync.dma_start(t[:], xT[3, hc])
                xt3.append(t)
            nc.sync.dma_start(wo_sb[:], wo[:])
            emit_A(3, xt3, extra_pe=tr2)
            tr3 = emit_B(3)

            # =========== Phase C: attention + wo (pipelined) ===========
            prev = None  # (q0, attn_map) of previous ab awaiting wo

            def wo_filler_gen(q0p, attn_map):
                """Yields once per call: emits one wo matmul; every 4th
                call closes a (qs, ht) chunk with copy + DMA out."""
                for qs in range(4):
                    for ht in range(4):
                        wop = t_wop("wop")
                        for h in range(NH):
                            nc.tensor.matmul(
                                wop[:],
                                attn_map[h][:, qs * P:(qs + 1) * P],
                                wo_sb[:, h * HID + ht * 512:
                                      h * HID + (ht + 1) * 512],
                                start=(h == 0), stop=(h == NH - 1))
                            yield
                        ot = outp.tile([P, 512], F32, name="ot", tag="ot")
                        nc.vector.tensor_copy(ot[:], wop[:])
                        nc.sync.dma_start(
                            out[q0p + qs * P: q0p + (qs + 1) * P,
                                ht * 512:(ht + 1) * 512], ot[:])

            for ab in range(NAB):
                q0 = ab * AB
                filler = (wo_filler_gen(*prev) if prev is not None else None)
                attn_map = {}
                for h in range(NH):
                    attps = t_catt(f"attps{h}")
                    sumps = t_csum(f"sumps{h}")
                    for kt in range(KC):
                        st = t_st("st")
                        nc.tensor.matmul(st[:],
                                         kT_sb[:, kt * P:(kt + 1) * P],
                                         xqT_list[h][:, q0:q0 + AB],
                                         start=True, stop=True)
                        e = esbp.tile([P, AB], BF16, name="e", tag="e")
                        nc.scalar.activation(e[:], st[:], AF.Exp)
                        nc.tensor.matmul(sumps[:], ones_b[:], e[:],
                                         start=(kt == 0), stop=(kt == KC - 1))
                        nc.tensor.matmul(attps[:],
                                         v_sb[:, kt * D:(kt + 1) * D],
                                         e[:],
                                         start=(kt == 0), stop=(kt == KC - 1))
                        if ab == 0 and h == 0 and kt < 4:
                            tr3(kt)  # deferred V-transposes of kcol 3
                        if filler is not None:
                            next(filler, None)
                    rr = small.tile([P, AB], F32, name="arr", tag="arr")
                    nc.vector.reciprocal_approx_fast(out=rr[:], in_=sumps[:])
                    attn = small.tile([P, AB], BF16, name="attn",
                                      tag=f"attn{h}", bufs=2)
                    nc.vector.tensor_tensor(out=attn[:], in0=attps[:],
                                            in1=rr[:], op=OP.mult)
                    attn_map[h] = attn
                if filler is not None:
                    for _ in filler:  # drain any remainder
                        pass
                prev = (q0, attn_map)

            # final ab's wo (no following block to interleave into)
            for _ in wo_filler_gen(*prev):
                pass

    nc.compile()
    return nc


def _get_compiled():
    global _compiled
    if _compiled is None:
        _compiled = _build()
    return _compiled


def _to_bf16_tiles(aT):
    """[HID, S] f32 -> [NPB, HC, 128, 512] bf16 contiguous."""
    t = aT.reshape(HC, P, NPB, PB).transpose(2, 0, 1, 3)
    return np.ascontiguousarray(t.astype(ml_dtypes.bfloat16))


def _weight_image(w, ncols):
    """[HC*P rows, ncols] -> SBUF image [128, HC*ncols] bf16."""
    nchunk = w.shape[0] // P
    img = w.reshape(nchunk, P, ncols).transpose(1, 0, 2).reshape(P, nchunk * ncols)
    return np.ascontiguousarray(img.astype(ml_dtypes.bfloat16))


def _shard_inputs(x, c, wq, wkv, wo, norm_q_w, norm_k_w):
    x = np.asarray(x, np.float32)
    c = np.asarray(c, np.float32)
    wq = np.asarray(wq, np.float32)
    wkv = np.asarray(wkv, np.float32)
    wo = np.asarray(wo, np.float32)
    nqw = (np.asarray(norm_q_w, np.float32) * np.float32(SCALE)).reshape(P, 1)
    nkw = np.asarray(norm_k_w, np.float32).reshape(P, 1).copy()

    xTs = [_to_bf16_tiles(x[b].T) for b in range(B)]
    cTs = [_to_bf16_tiles(c[b].T) for b in range(B)]
    in_maps = []
    for core in range(8):
        b, g = core // 4, core % 4
        blk = wkv[:, g * 256:(g + 1) * 256]
        wq_sh = wq[:, g * 512:(g + 1) * 512]
        in_maps.append({
            "xT": xTs[b],
            "cT": cTs[b],
            "wq": np.ascontiguousarray(
                wq_sh.reshape(HC, P, NH * D).astype(ml_dtypes.bfloat16)),
            "wk": _weight_image(np.ascontiguousarray(blk[:, 0::2]), D),
            "wv": _weight_image(np.ascontiguousarray(blk[:, 1::2]), D),
            "wo": _weight_image(wo[g * 512:(g + 1) * 512, :], HID),
            "nqw": nqw,
            "nkw": nkw,
        })
    return in_maps


def run_sharded(inputs, trace=False, trace_cores=None):
    """Run the SPMD kernel; returns (full_output, BassKernelResults)."""
    nc = _get_compiled()
    in_maps = _shard_inputs(**inputs)
    res = run_bass_kernel_spmd(nc, in_maps, core_ids=list(range(8)),
                               trace=trace, trace_cores=trace_cores)
    parts = [r["out"] for r in res.results]
    full = np.empty((B, S, HID), np.float32)
    for b in range(B):
        full[b] = np.sum(np.stack([parts[4 * b + g] for g in range(4)], 0),
                         axis=0, dtype=np.float64).astype(np.float32)
    return full, res


def kernel(**inputs) -> np.ndarray:
    out, _ = run_sharded(inputs, trace=False)
    return out
